# revision 7
# baseline (speedup 1.0000x reference)
"""Trainium2 Bass kernel for nn_Attention_558345749040.

Reference (per batch b, H=8 heads of d=64, S=4096, E=512):
    Q = Q_seq @ WQ ; K = K_seq @ WK ; V = V_seq @ WV
    A = (Q * K) / 8                       (elementwise logits, [S, H, d])
    A[j >= V_len[b]] -= 1e12              (additive mask over head features)
    softmax over each head's d-wide group
    O = softmax * V, rows s >= Q_len[b] zeroed.

Key structural facts exploited (all data-dependent work is derived at
runtime from Q_len/V_len and baked into a compiled schedule, cached per
schedule signature, so the kernel stays correct for arbitrary inputs):

 1. Rows s >= Q_len[b] are zero by definition -> never loaded/computed.
    Live tokens are packed into per-core streams host-side.
 2. Softmax spans only j < V_len[b] per head; output columns beyond are
    zero.  Projections stream only the first 8*pad8(V_len) packed weight
    columns (3D access pattern on shared fp16 weights); the elementwise
    chain runs at the packed width.
 3. V_len == 0 collapses the reference to O = V/64 (uniform softmax over
    the fully-masked group): V-projection-only slots, no Q/K work.
 4. fp16 matmuls (full-rate PE + FWL weight loads + half DMA traffic);
    PSUM accumulates in f32.  Validated rel err ~4e-3 vs the 2e-2 gate.
 5. Masking happens on-device from per-core (vlen, iota) data via one
    additive -1e9 pass: masked logits ride exp() to exactly 0, so the
    denominator needs no separate masked-multiply pass.

Work unit: a "pair" = 256 tokens (2 chunks of 128 partitions).  Pairs are
sorted by packed width and dealt 8-at-a-time into slots; every core runs
the same static program whose slot k uses the max width in slot k's deal
group (sorting makes the 8 widths near-equal).  V-only pairs form their
own slots at the tail.
"""

import numpy as np
import ml_dtypes

B, S, EMB = 8, 4096, 512
H, D = 8, 64
NCORES = 8
KC = EMB // 128          # 4 contraction chunks
PTOK = 256               # tokens per pair/slot
_CACHE = {}


def _pad8(x):
    return (int(x) + 7) // 8 * 8


def _schedule(Q_len, V_len):
    """Build the (slot widths, deal) schedule from the length tensors."""
    qk_pairs, v_pairs = [], []
    for b in range(B):
        ql = int(Q_len[b, 0])
        vl = int(V_len[b, 0])
        npair = -(-ql // PTOK)
        for i in range(npair):
            if vl == 0:
                v_pairs.append((b, i * PTOK))
            else:
                qk_pairs.append((b, i * PTOK, 8 * _pad8(vl), vl))
    qk_pairs.sort(key=lambda p: -p[2])
    nqk = -(-len(qk_pairs) // NCORES)
    while len(qk_pairs) < nqk * NCORES:
        qk_pairs.append(None)                      # dummy work
    nv = -(-len(v_pairs) // NCORES)
    while len(v_pairs) < nv * NCORES:
        v_pairs.append(None)
    slot_w = []
    for k in range(nqk):
        grp = [p for p in qk_pairs[NCORES * k:NCORES * (k + 1)] if p]
        slot_w.append(max(p[2] for p in grp) if grp else 128)
    deal_qk = [[qk_pairs[NCORES * k + c] for k in range(nqk)]
               for c in range(NCORES)]
    deal_v = [[v_pairs[NCORES * k + c] for k in range(nv)]
              for c in range(NCORES)]
    return tuple(slot_w), deal_qk, deal_v, nv


def _build(slot_w, nv):
    import concourse.bacc as bacc
    import concourse.mybir as mybir
    from concourse.tile import TileContext

    f32 = mybir.dt.float32
    f16 = mybir.dt.float16
    bf16 = mybir.dt.bfloat16
    AX = mybir.AxisListType
    OP = mybir.AluOpType
    ACTF = mybir.ActivationFunctionType

    nqk = len(slot_w)
    T_qk = nqk * PTOK
    T_all = (nqk + nv) * PTOK
    dset = sorted({w // 8 for w in slot_w})        # distinct group sizes
    iota_off = {}
    c = 0
    for d in dset:
        iota_off[d] = c
        c += 8 * d
    iota_cols = max(c, 1)

    nc = bacc.Bacc()

    qT = kT = iotas = vlens = None
    if nqk:
        qT = nc.declare_dram_parameter("qT", [EMB, T_qk], f16, isOutput=False)
        kT = nc.declare_dram_parameter("kT", [EMB, T_qk], f16, isOutput=False)
        iotas = nc.declare_dram_parameter("iotas", [128, iota_cols], f32,
                                          isOutput=False)
        vlens = nc.declare_dram_parameter("vlens", [128, nqk], f32,
                                          isOutput=False)
    vT = nc.declare_dram_parameter("vT", [EMB, T_all], f16, isOutput=False)
    wq = nc.declare_dram_parameter("wq", [EMB, EMB], f16, isOutput=False)
    wk = nc.declare_dram_parameter("wk", [EMB, EMB], f16, isOutput=False)
    wv = nc.declare_dram_parameter("wv", [EMB, EMB], f16, isOutput=False)
    out = nc.declare_dram_parameter("out", [T_all, EMB], bf16, isOutput=True)

    # DMA supers: groups of <=4 slots (2KB/partition lines); slot 0 alone
    # so the first matmul isn't gated on a 3MB transfer.
    supers = []
    k = 0
    first = True
    while k < nqk + nv:
        n = 1 if first else min(4, nqk + nv - k)
        # don't mix qk and v slots in one super
        if k < nqk:
            n = min(n, nqk - k)
        supers.append((k, n))
        first = False
        k += n
    sup_of = {}
    for si, (k0, n) in enumerate(supers):
        for k in range(k0, k0 + n):
            sup_of[k] = si

    with TileContext(nc) as tc:
        with (
            tc.tile_pool(name="consts", bufs=1) as cpool,
            tc.tile_pool(name="xin", bufs=2) as xpool,
            tc.tile_pool(name="psq2", bufs=2, space="PSUM") as qpool,
            tc.tile_pool(name="psk2", bufs=2, space="PSUM") as kpool,
            tc.tile_pool(name="psv3", bufs=3, space="PSUM") as vpool,
            tc.tile_pool(name="work", bufs=3) as wpool,
            tc.tile_pool(name="stats", bufs=4) as spool,
        ):
            # ---- constants -------------------------------------------------
            w_sb = {}
            for name, src in (("wq", wq), ("wk", wk), ("wv", wv)):
                tiles = []
                for kc in range(KC):
                    t = cpool.tile([128, EMB], f16, tag=f"{name}{kc}",
                                   name=f"{name}{kc}")
                    nc.sync.dma_start(out=t[:],
                                      in_=src[kc * 128:(kc + 1) * 128, :])
                    tiles.append(t)
                w_sb[name] = tiles

            vmadds = []
            if nqk:
                iot_sb = cpool.tile([128, iota_cols], f32, tag="iotas")
                nc.sync.dma_start(out=iot_sb[:], in_=iotas[:, :])
                vl_sb = cpool.tile([128, nqk], f32, tag="vlens")
                nc.sync.dma_start(out=vl_sb[:], in_=vlens[:, :])
                # per-slot additive mask rows [128, w]: -1e9 where j >= vlen
                for k in range(nqk):
                    w = slot_w[k]
                    d = w // 8
                    vmadd = cpool.tile([128, 512], f32, tag=f"vmadd{k}",
                                       name=f"vmadd{k}")
                    io = iota_off[d]
                    nc.vector.tensor_scalar(
                        vmadd[:, :w], iot_sb[:, io:io + w],
                        vl_sb[:, k:k + 1], -1e9,
                        OP.is_ge, OP.mult,
                    )
                    vmadds.append(vmadd)

            # weight views: packed (h, j<d) columns, one per distinct d
            def wview(name, kc, d):
                ap = w_sb[name][kc][:].rearrange("p (h j) -> p h j", j=D)
                return ap[:, :, :d]

            def load_super(si):
                k0, n = supers[si]
                cols = n * PTOK
                xs = {}
                names = ("q", "k", "v") if k0 < nqk else ("v",)
                for name in names:
                    src = {"q": qT, "k": kT, "v": vT}[name]
                    xs[name] = []
                    for kc in range(KC):
                        t = xpool.tile([128, 4 * PTOK], f16, tag=f"x{name}{kc}",
                                       name=f"x{name}{kc}")
                        nc.sync.dma_start(
                            out=t[:, :cols],
                            in_=src[kc * 128:(kc + 1) * 128,
                                    k0 * PTOK:k0 * PTOK + cols],
                        )
                        xs[name].append(t)
                return xs

            def mm_group(ps_ap, xtiles, wname, js, d):
                for kc in range(KC):
                    nc.tensor.matmul(
                        ps_ap,
                        xtiles[kc][:, js],
                        wview(wname, kc, d),
                        start=(kc == 0),
                        stop=(kc == KC - 1),
                    )

            def front_qk(k, xs):
                w = slot_w[k]
                d = w // 8
                k0 = supers[sup_of[k]][0]
                a = wpool.tile([128, 1024], f32, tag="a")
                am = wpool.tile([128, 1024], f32, tag="am")
                t_m = wpool.tile([128, 1024], f32, tag="t_m")
                k_sb = wpool.tile([128, 1024], f32, tag="k_sb")
                e = wpool.tile([128, 1024], bf16, tag="e")
                u = wpool.tile([128, 1024], bf16, tag="u")
                for c in range(2):
                    js = slice((k - k0) * PTOK + c * 128,
                               (k - k0) * PTOK + (c + 1) * 128)
                    cs = slice(c * w, (c + 1) * w)
                    psq = qpool.tile([128, EMB], f32, tag="psq")
                    psk = kpool.tile([128, EMB], f32, tag="psk")
                    pq = psq[:, :w].rearrange("p (h j) -> p h j", j=d)
                    pk = psk[:, :w].rearrange("p (h j) -> p h j", j=d)
                    mm_group(pk, xs["k"], "wk", js, d)
                    mm_group(pq, xs["q"], "wq", js, d)
                    # only one PSUM operand allowed per DVE op: stage K
                    # through SBUF on the (underloaded) scalar engine
                    nc.scalar.copy(k_sb[:, cs], psk[:, :w])
                    nc.vector.tensor_mul(a[:, cs], psq[:, :w], k_sb[:, cs])
                def g3(ap):
                    return ap[:, :2 * w].rearrange("p (g j) -> p g j", j=d)
                # additive mask (gpsimd): masked cols -> -1e9
                vmadd_bc = vmadds[k][:, :w].rearrange(
                    "p (o f) -> p o f", o=1).broadcast_to((128, 2, w))
                a2 = a[:, :2 * w].rearrange("p (o f) -> p o f", o=2)
                am2 = am[:, :2 * w].rearrange("p (o f) -> p o f", o=2)
                nc.gpsimd.tensor_add(am2, a2, vmadd_bc)
                # per-(chunk, head) max, negated
                mneg = spool.tile([128, 16], f32, tag="mneg")
                nc.vector.tensor_reduce(mneg[:], g3(am), axis=AX.X,
                                        op=OP.max, negate=True)
                mneg_bc = mneg[:].rearrange(
                    "p (g o) -> p g o", o=1).broadcast_to((128, 16, d))
                nc.vector.tensor_add(g3(t_m), g3(am), mneg_bc)
                nc.scalar.activation(e[:, :2 * w], t_m[:, :2 * w], ACTF.Exp)
                for c in range(2):
                    js = slice((k - k0) * PTOK + c * 128,
                               (k - k0) * PTOK + (c + 1) * 128)
                    cs = slice(c * w, (c + 1) * w)
                    psv = vpool.tile([128, EMB], f32, tag="psv")
                    pv = psv[:, :w].rearrange("p (h j) -> p h j", j=d)
                    mm_group(pv, xs["v"], "wv", js, d)
                    # weight by V straight out of PSUM (gpsimd has no
                    # PSUM port; vector may read one PSUM operand)
                    nc.vector.tensor_mul(u[:, cs], e[:, cs], psv[:, :w])
                return e, u, w, d

            def back_qk(k, e, u, w, d):
                def g3(ap):
                    return ap[:, :2 * w].rearrange("p (g j) -> p g j", j=d)
                ssum = spool.tile([128, 16], f32, tag="ssum")
                nc.vector.tensor_reduce(ssum[:], g3(e), axis=AX.X, op=OP.add)
                r = spool.tile([128, 16], bf16, tag="r")
                with nc.allow_low_precision(reason="1/S at bf16: ~0.4% on "
                                            "softmax weights vs 2e-2 gate"):
                    nc.vector.reciprocal(r[:], ssum[:])
                r_bc = r[:].rearrange(
                    "p (g o) -> p g o", o=1).broadcast_to((128, 16, d))
                o = wpool.tile([128, 1024], bf16, tag="o")
                nc.gpsimd.tensor_mul(g3(o), g3(u), r_bc)
                t0 = k * PTOK
                nc.sync.dma_start(
                    out=out[t0:t0 + PTOK, :w].rearrange(
                        "(i p) f -> p i f", i=2),
                    in_=o[:, :2 * w].rearrange("p (i f) -> p i f", i=2),
                )

            def slot_v(k, xs):
                k0 = supers[sup_of[k]][0]
                vsb = wpool.tile([128, 1024], bf16, tag="o")
                for c in range(2):
                    js = slice((k - k0) * PTOK + c * 128,
                               (k - k0) * PTOK + (c + 1) * 128)
                    psv = vpool.tile([128, EMB], f32, tag="psv")
                    mm_group(psv[:].rearrange("p (h j) -> p h j", j=D),
                             xs["v"], "wv", js, D)
                    nc.scalar.activation(vsb[:, c * EMB:(c + 1) * EMB],
                                         psv[:], ACTF.Copy, scale=1.0 / 64)
                t0 = k * PTOK
                nc.sync.dma_start(
                    out=out[t0:t0 + PTOK, :].rearrange(
                        "(i p) f -> p i f", i=2),
                    in_=vsb[:].rearrange("p (i f) -> p i f", i=2),
                )

            xs_cur = load_super(0) if supers else None
            pending = None
            nslots = nqk + nv
            for k in range(nslots + 1):
                if k < nslots:
                    si = sup_of[k]
                    if k == supers[si][0] and k > 0:
                        xs_cur = load_super(si)
                    if k < nqk:
                        front = front_qk(k, xs_cur)
                    else:
                        slot_v(k, xs_cur)
                        front = None
                else:
                    front = None
                if pending is not None:
                    back_qk(k - 1, *pending)
                pending = front

    nc.finalize()
    return nc


def _prep_inputs(Q_seq, K_seq, V_seq, Q_len, V_len, WQ, WK, WV,
                 slot_w, deal_qk, deal_v, nv):
    f16 = np.float16
    nqk = len(slot_w)
    dset = sorted({w // 8 for w in slot_w})
    iota_cols = 0
    iota_rows = []
    for d in dset:
        iota_cols += 8 * d
        iota_rows.append(np.tile(np.arange(d, dtype=np.float32), 8))
    iota = (np.broadcast_to(np.concatenate(iota_rows), (128, iota_cols))
            .copy() if dset else None)

    wq_h = np.ascontiguousarray((WQ * 0.125).astype(f16))
    wk_h = np.ascontiguousarray(WK.astype(f16))
    wv_h = np.ascontiguousarray(WV.astype(f16))

    qTs = {b: np.ascontiguousarray(Q_seq[b].T.astype(f16)) for b in range(B)}
    kTs = {b: np.ascontiguousarray(K_seq[b].T.astype(f16)) for b in range(B)}
    vTs = {b: np.ascontiguousarray(V_seq[b].T.astype(f16)) for b in range(B)}

    in_maps = []
    for c in range(NCORES):
        qcols, kcols, vcols, vl_row = [], [], [], []
        for k in range(nqk):
            p = deal_qk[c][k]
            if p is None:
                qcols.append(np.zeros((EMB, PTOK), f16))
                kcols.append(np.zeros((EMB, PTOK), f16))
                vcols.append(np.zeros((EMB, PTOK), f16))
                vl_row.append(8.0)
            else:
                b, t0, w, vl = p
                qcols.append(qTs[b][:, t0:t0 + PTOK])
                kcols.append(kTs[b][:, t0:t0 + PTOK])
                vcols.append(vTs[b][:, t0:t0 + PTOK])
                vl_row.append(float(vl))
        for k in range(nv):
            p = deal_v[c][k]
            if p is None:
                vcols.append(np.zeros((EMB, PTOK), f16))
            else:
                b, t0 = p
                vcols.append(vTs[b][:, t0:t0 + PTOK])
        m = {
            "vT": (np.concatenate(vcols, axis=1) if vcols
                   else np.zeros((EMB, 0), f16)),
            "wq": wq_h, "wk": wk_h, "wv": wv_h,
        }
        if nqk:
            m["qT"] = np.concatenate(qcols, axis=1)
            m["kT"] = np.concatenate(kcols, axis=1)
            m["iotas"] = iota
            m["vlens"] = np.broadcast_to(
                np.asarray(vl_row, np.float32), (128, nqk)).copy()
        in_maps.append(m)
    return in_maps


def _unpack(res, Q_len, V_len, slot_w, deal_qk, deal_v, nv):
    nqk = len(slot_w)
    full = np.zeros((B, S, EMB), np.float32)
    for c in range(NCORES):
        o = np.asarray(res.results[c]["out"]).astype(np.float32)
        for k in range(nqk):
            p = deal_qk[c][k]
            if p is None:
                continue
            b, t0, w, vl = p
            d = slot_w[k] // 8
            nrow = min(PTOK, int(Q_len[b, 0]) - t0)
            blk = o[k * PTOK:k * PTOK + nrow, :8 * d].reshape(nrow, 8, d)
            full[b, t0:t0 + nrow].reshape(nrow, H, D)[:, :, :vl] = \
                blk[:, :, :vl]
        for k in range(nv):
            p = deal_v[c][k]
            if p is None:
                continue
            b, t0 = p
            nrow = min(PTOK, int(Q_len[b, 0]) - t0)
            full[b, t0:t0 + nrow] = o[(nqk + k) * PTOK:
                                      (nqk + k) * PTOK + nrow, :]
    return full


def _run(inputs, trace=False, mm_dtype_name="", tmpdir=None):
    from concourse.bass_utils import run_bass_kernel_spmd

    Q_len = np.asarray(inputs["Q_len"])
    V_len = np.asarray(inputs["V_len"])
    slot_w, deal_qk, deal_v, nv = _schedule(Q_len, V_len)

    if len(slot_w) == 0 and nv == 0:
        return np.zeros((B, S, EMB), np.float32), None

    key = (slot_w, nv)
    if key not in _CACHE:
        _CACHE[key] = _build(slot_w, nv)
    nc = _CACHE[key]

    in_maps = _prep_inputs(
        np.asarray(inputs["Q_seq"]), np.asarray(inputs["K_seq"]),
        np.asarray(inputs["V_seq"]), Q_len, V_len,
        np.asarray(inputs["WQ"]), np.asarray(inputs["WK"]),
        np.asarray(inputs["WV"]), slot_w, deal_qk, deal_v, nv)
    res = run_bass_kernel_spmd(nc, in_maps, core_ids=list(range(NCORES)),
                               trace=trace, tmpdir=tmpdir)
    out = _unpack(res, Q_len, V_len, slot_w, deal_qk, deal_v, nv)
    return out, res


def kernel(Q_seq, K_seq, V_seq, Q_len, V_len, WQ, WK, WV):
    out, _ = _run(dict(Q_seq=Q_seq, K_seq=K_seq, V_seq=V_seq,
                       Q_len=Q_len, V_len=V_len, WQ=WQ, WK=WK, WV=WV))
    return out
